# revision 1
# baseline (speedup 1.0000x reference)
"""Distributed causal multi-head attention for 8 TRN2 NeuronCores.

Problem: x[4,2048,512], 8 heads, causal. out = Attn(x) @ Wo.T + bo.

Sharding: 2 cores per batch element. Each core computes 1024 query rows of
its batch, as four 256-row panels at positions p=0..3 with k-extent
512*(p+1). Core (b,0) takes rows [512p, 512p+256), core (b,1) takes
[512p+256, 512p+512); both run the identical SPMD graph - all per-core
differences flow through input data (gathered query rows, mask tiles).

Device layouts are transposed so that per-query softmax reductions become
matmuls / ones-column tricks instead of partition reductions:
  QT[j,q], KT[j,k] from  W.T @ x.T ;  V[k,j] natural;
  S^T[k,q] = KT_head.T @ QT_head (heads on partitions 0-63);
  P = exp(S^T/8) on ScalarE, causal mask as a DVE multiply (2x mode);
  o^T[d,q] accumulated over k-blocks with two heads col-packed in PSUM;
  softmax denominator rides along as a ones-column of V.

Performance structure (303us -> ~151us on HW):
  - K^T/Q^T head-PAIR-stacked on 128 partitions: the two heads' K=64
    score matmuls run concurrently as PE row tiles (0,0)/(64,0), ~2x
    score throughput. Their outputs must land in different PSUM banks
    (hh-major s layout) - concurrent same-bank drains are device-fatal.
  - all DRAM inputs pre-packed host-side so every dma_start is a fully
    contiguous [128, bytes] block; wk / x chunk 0 split by contraction
    block so the first matmul starts ~8us in.
  - chunk-pipelined and interleaved: projection matmul groups for k-chunk
    kc+1 are emitted between attention batches of panel kc, so PE always
    has dense ready work while ScalarE runs exp. This keeps the PE HAM
    clock-gate at 2.4 GHz for the whole kernel (idle gaps re-throttle it
    to 1.2 GHz, which doubled every matmul in earlier versions).
  - K bias dropped (adds a per-q constant to scores -> cancels in
    softmax); V bias folded host-side into bo' = bo + bv @ Wo.T.
  - head-split of K/Q projections via DVE copies out of PSUM; V copies
    on ScalarE to balance engine load.
  - the normalize chain (den copy -> recip -> gpsimd partition_broadcast
    of 1/den -> muls) of head pair hp is emitted inside head pair hp+1's
    score stream, so the in-order PE queue never head-of-line blocks on
    the DVE chain. reciprocal_approx_fast must read SBUF, not PSUM (PSUM
    input returns garbage on HW even though CoreSim accepts it).
  - O-projection packs head pairs on 128 partitions (K=128 contraction),
    deferred behind the next panel's scores; the last panel accumulates
    it per head pair so the kernel tail is one matmul pair.
"""

import os
import sys

import numpy as np

sys.path.insert(0, "/opt/trn_rl_repo")

import concourse.bass as bass  # noqa: E402
import concourse.mybir as mybir  # noqa: E402
from concourse import bacc, library_config  # noqa: E402
from concourse.tile import TileContext  # noqa: E402

P = 128
D = 512
S = 2048
H = 8
DH = 64
NPANEL = 4
QP = 256  # query rows per panel
NQ = NPANEL * QP  # 1024 query rows per core
SCALE = 0.125  # 1/sqrt(DH)

MMDT_NAME = os.environ.get("KERNEL_MMDT", "bf16")
MASK_GS = os.environ.get("KERNEL_MASK_GS", "0") == "1"
PVDEPTH = int(os.environ.get("KERNEL_PVDEPTH", "2"))
PB_GS = os.environ.get("KERNEL_PB_GS", "1") == "1"
VCOPY_ACT = os.environ.get("KERNEL_VCOPY_ACT", "1") == "1"

f32 = mybir.dt.float32
Exp = mybir.ActivationFunctionType.Exp
add_op = mybir.AluOpType.add
mult_op = mybir.AluOpType.mult

MMDT = {"bf16": mybir.dt.bfloat16, "f32r": mybir.dt.float32r, "f32": f32}[MMDT_NAME]


def build():
    # Bacc (not Bass): its compile() pipeline runs generate_event_semaphores,
    # which splits multi-wait instructions to satisfy the 1-wait-per-
    # instruction hardware limit.
    nc = bacc.Bacc()

    xTp = nc.declare_dram_parameter("xTp", [4, P, 4, 512], MMDT, isOutput=False)
    xqTp = nc.declare_dram_parameter("xqTp", [2, P, 4, 512], MMDT, isOutput=False)
    wqp = nc.declare_dram_parameter("wqp", [P, 4, D], MMDT, isOutput=False)
    wkp = nc.declare_dram_parameter("wkp", [4, P, D], MMDT, isOutput=False)
    wvp = nc.declare_dram_parameter("wvp", [P, 4, D], MMDT, isOutput=False)
    woT2 = nc.declare_dram_parameter("woT2", [P, 4, D], MMDT, isOutput=False)
    bq = nc.declare_dram_parameter("bq", [P, 4], f32, isOutput=False)
    bo_bc = nc.declare_dram_parameter("bo_bc", [P, D], f32, isOutput=False)
    maskp = nc.declare_dram_parameter("maskp", [NPANEL, P, 4, 2, QP], MMDT, isOutput=False)
    ones64 = nc.declare_dram_parameter("ones64", [1, DH], MMDT, isOutput=False)
    out = nc.declare_dram_parameter("out", [NQ, D], f32, isOutput=True)

    with nc.allow_low_precision(reason="bf16 matmul operands"), TileContext(nc) as tc:
        with (
            tc.tile_pool(name="big", bufs=1) as bpool,
            tc.tile_pool(name="attp", bufs=2) as apool,
            tc.tile_pool(name="work", bufs=PVDEPTH + 2) as wpool,
            tc.tile_pool(name="osb", bufs=2) as opool,
            tc.tile_pool(name="ps_proj", bufs=2, space="PSUM") as ps_proj,
            tc.tile_pool(name="ps_s", bufs=2, space="PSUM") as ps_s,
            tc.tile_pool(name="ps_ot", bufs=2, space="PSUM") as ps_ot,
        ):
            # ---- persistent SBUF tensors ----
            xT_sb = bpool.tile([P, 4, 4, 512], MMDT, tag="xT")
            xqT_sb = bpool.tile([P, 2, 4, 512], MMDT, tag="xqT")
            # K^T/Q^T stored head-PAIR-stacked: head 2hp on partitions
            # 0-63, head 2hp+1 on 64-127. The two heads' score matmuls run
            # CONCURRENTLY as PE row tiles (0,0)/(64,0); their outputs go
            # to different PSUM banks (hh-major s layout) to avoid
            # concurrent same-bank drains.
            kT_sb = bpool.tile([P, 4, S], MMDT, tag="kT")
            v_sb = bpool.tile([P, S // P, H, DH + 1], MMDT, tag="v")
            qT_sb = bpool.tile([P, 4, NQ], MMDT, tag="qT")
            w_sb = {}
            for name in ("wk", "wq", "wv"):
                w_sb[name] = bpool.tile([P, 4, D], MMDT, tag=name, name=name)
            wo_sb = bpool.tile([P, 4, D], MMDT, tag="wo")
            bq_sb = bpool.tile([P, 4], f32, tag="bq")
            bo_sb = bpool.tile([P, D], f32, tag="bo")
            mask_sb = bpool.tile([P, NPANEL, 4, 2, QP], MMDT, tag="mask")
            # ones64/o64 only feed the PB_GS=0 fallback; skip the DMA on the
            # default path so the x chunk-1 transfer lands earlier
            o64_sb = None
            if not PB_GS:
                o64_sb = bpool.tile([1, DH], MMDT, tag="o64")

            # input DMAs in consumption order; every transfer is contiguous.
            # wk and x chunk 0 are split by db so the first K-projection
            # matmul can start after ~256KB instead of ~1MB of DMA.
            for db in range(4):
                nc.sync.dma_start(out=w_sb["wk"][:, db, :], in_=wkp[db])
                nc.sync.dma_start(out=xT_sb[:, 0, db, :], in_=xTp[0, :, db, :])
            nc.sync.dma_start(out=xqT_sb[:, 0], in_=xqTp[0])
            nc.sync.dma_start(out=w_sb["wq"][:], in_=wqp[:])
            nc.sync.dma_start(out=bq_sb[:], in_=bq[:])
            nc.sync.dma_start(out=w_sb["wv"][:], in_=wvp[:])
            nc.sync.dma_start(out=mask_sb[:, 0], in_=maskp[0])
            if not PB_GS:
                nc.sync.dma_start(out=o64_sb[:], in_=ones64[:])
            nc.sync.dma_start(out=xT_sb[:, 1], in_=xTp[1])
            nc.sync.dma_start(out=wo_sb[:], in_=woT2[:])
            nc.sync.dma_start(out=bo_sb[:], in_=bo_bc[:])
            nc.sync.dma_start(out=mask_sb[:, 1], in_=maskp[1])
            nc.sync.dma_start(out=xT_sb[:, 2], in_=xTp[2])
            nc.sync.dma_start(out=xqT_sb[:, 1], in_=xqTp[1])
            nc.sync.dma_start(out=mask_sb[:, 2], in_=maskp[2])
            nc.sync.dma_start(out=xT_sb[:, 3], in_=xTp[3])
            nc.sync.dma_start(out=mask_sb[:, 3], in_=maskp[3])
            # ones column appended per head so P.V also yields the softmax
            # denominator in psum row DH for free
            nc.vector.memset(v_sb[:, :, :, DH : DH + 1], 1.0)
            # gpsimd runs the mask multiplies and the 1/den partition
            # broadcast; partition_broadcast lives in the attn library
            nc.gpsimd.load_library(library_config.attn)

            deferred = {"norm": None, "oproj": None, "o_mm": None}

            def proj_chunk_gen(kc):
                """Yields after each matmul group so the caller can
                interleave projection work into the attention stream."""
                # K^T[j, k-chunk]; no bias (cancels in softmax); head-major
                # split via DVE copies
                for jb in range(4):
                    ps = ps_proj.tile([P, 512], f32, tag="p512")
                    for db in range(4):
                        nc.tensor.matmul(
                            ps[:],
                            lhsT=w_sb["wk"][:, db, jb * P : (jb + 1) * P],
                            rhs=xT_sb[:, kc, db, :],
                            start=(db == 0),
                            stop=(db == 3),
                        )
                    nc.vector.tensor_copy(
                        out=kT_sb[:, jb, kc * 512 : (kc + 1) * 512], in_=ps[:]
                    )
                    yield
                # Q^T for a 512-row half (two panels) on even chunks
                if kc % 2 == 0:
                    hf = kc // 2
                    for jb in range(4):
                        ps = ps_proj.tile([P, 512], f32, tag="p512")
                        for db in range(4):
                            nc.tensor.matmul(
                                ps[:],
                                lhsT=w_sb["wq"][:, db, jb * P : (jb + 1) * P],
                                rhs=xqT_sb[:, hf, db, :],
                                start=(db == 0),
                                stop=(db == 3),
                            )
                        nc.vector.tensor_tensor(
                            qT_sb[:, jb, hf * 512 : (hf + 1) * 512],
                            ps[:],
                            bq_sb[:, jb : jb + 1].to_broadcast([P, 512]),
                            add_op,
                        )
                        yield
                # V[k-chunk, j]; no bias (folded into bo' host-side)
                for kb in range(4):
                    ps = ps_proj.tile([P, 512], f32, tag="p512")
                    for db in range(4):
                        nc.tensor.matmul(
                            ps[:],
                            lhsT=xT_sb[:, kc, db, kb * P : (kb + 1) * P],
                            rhs=w_sb["wv"][:, db, :],
                            start=(db == 0),
                            stop=(db == 3),
                        )
                    if VCOPY_ACT:
                        nc.scalar.copy(
                            out=v_sb[:, 4 * kc + kb, :, 0:DH],
                            in_=ps[:].rearrange("p (h d) -> p h d", h=H),
                        )
                    else:
                        nc.vector.tensor_copy(
                            out=v_sb[:, 4 * kc + kb, :, 0:DH],
                            in_=ps[:].rearrange("p (h d) -> p h d", h=H),
                        )
                    yield

            def make_norm(hp, ot_ps, attT_sb, after=None, split_hh=False):
                def emit_norm():
                    # attT[:, hp pair, :] = ot / den; den sits in psum row DH.
                    # custom-DVE recip must read SBUF (PSUM input returns
                    # garbage on HW even though CoreSim accepts it).
                    den_sb = wpool.tile([1, 2, QP], f32, tag="den_sb")
                    rden_f = wpool.tile([1, 2, QP], f32, tag="rden_f")
                    bc_sb = wpool.tile([DH, 2, QP], f32, tag="bc_sb")
                    if split_hh:
                        # per-hh chains pipeline across DVE/GpSimd: lower
                        # latency for the kernel-tail normalize
                        for hh in range(2):
                            nc.vector.tensor_copy(
                                out=den_sb[:, hh, :],
                                in_=ot_ps[DH : DH + 1, hh, :],
                            )
                            nc.vector.reciprocal_approx_fast(
                                out=rden_f[:, hh, :], in_=den_sb[:, hh, :]
                            )
                            nc.gpsimd.partition_broadcast(
                                bc_sb[:, hh, :], rden_f[:, hh, :]
                            )
                    elif PB_GS:
                        nc.vector.tensor_copy(
                            out=den_sb[:], in_=ot_ps[DH : DH + 1, :, :]
                        )
                        nc.vector.reciprocal_approx_fast(
                            out=rden_f[:], in_=den_sb[:]
                        )
                        # broadcast 1/den across the 64 dh partitions on
                        # gpsimd (keeps PE/DVE out of the norm critical path)
                        nc.gpsimd.partition_broadcast(bc_sb[:], rden_f[:])
                    else:
                        nc.vector.tensor_copy(
                            out=den_sb[:], in_=ot_ps[DH : DH + 1, :, :]
                        )
                        nc.vector.reciprocal_approx_fast(
                            out=rden_f[:], in_=den_sb[:]
                        )
                        rden = wpool.tile([1, 2, QP], MMDT, tag="rden")
                        nc.vector.tensor_copy(out=rden[:], in_=rden_f[:])
                        bc_full = ps_s.tile([P, 2, 2, QP], f32, tag="s", name="bc")
                        bc_ps = bc_full[0:DH, 0, :, :]
                        nc.tensor.matmul(
                            bc_ps, lhsT=o64_sb[:], rhs=rden[:], start=True, stop=True
                        )
                        nc.vector.tensor_copy(out=bc_sb[:], in_=bc_ps)
                    for hh in range(2):
                        nc.vector.tensor_mul(
                            out=attT_sb[hh * DH : (hh + 1) * DH, hp, :],
                            in0=ot_ps[0:DH, hh, :],
                            in1=bc_sb[:, hh, :],
                        )
                    if after is not None:
                        after()

                return emit_norm

            # last panel: O-projection accumulates per head pair as each
            # norm completes, so the kernel tail is one matmul pair instead
            # of eight matmuls behind the final norm
            last_ps = {}

            def make_o_mm(p, hp, attT_sb):
                def emit():
                    for qs in range(2):
                        if hp == 0:
                            last_ps[qs] = ps_proj.tile([P, D], f32, tag="p512", name=f"lastps{qs}")
                        nc.tensor.matmul(
                            last_ps[qs][:],
                            lhsT=attT_sb[:, hp, qs * P : (qs + 1) * P],
                            rhs=wo_sb[:, hp, :],
                            start=(hp == 0),
                            stop=(hp == 3),
                        )
                        if hp == 3:
                            osb = opool.tile([P, D], f32, tag="osb")
                            nc.vector.tensor_tensor(osb[:], last_ps[qs][:], bo_sb[:], add_op)
                            nc.sync.dma_start(
                                out=out[p * QP + qs * P : p * QP + (qs + 1) * P, :],
                                in_=osb[:],
                            )

                return emit

            def make_oproj(p, attT_sb):
                def emit_oproj():
                    # out[q,:] = attT.T @ Wo.T + bo'; head pairs packed so the
                    # contraction uses all 128 partitions
                    for qs in range(2):
                        ps = ps_proj.tile([P, D], f32, tag="p512")
                        for hp in range(4):
                            nc.tensor.matmul(
                                ps[:],
                                lhsT=attT_sb[:, hp, qs * P : (qs + 1) * P],
                                rhs=wo_sb[:, hp, :],
                                start=(hp == 0),
                                stop=(hp == 3),
                            )
                        osb = opool.tile([P, D], f32, tag="osb")
                        nc.vector.tensor_tensor(osb[:], ps[:], bo_sb[:], add_op)
                        nc.sync.dma_start(
                            out=out[p * QP + qs * P : p * QP + (qs + 1) * P, :],
                            in_=osb[:],
                        )

                return emit_oproj

            def emit_attention_panel(p, gen):
                nblk = 4 * (p + 1)
                nbat = nblk // 2  # 2 k-blocks per exp batch
                q0 = p * QP
                norm_at = 1 if nbat == 2 else 2
                attT_sb = apool.tile([P, 4, QP], MMDT, tag="attT")
                for hp in range(4):  # head pairs (2hp, 2hp+1)
                    ot_ps = ps_ot.tile([DH + 1, 2, QP], f32, tag="ot")

                    def emit_pv(bb, pT, hp=hp, nbat=nbat, ot_ps=ot_ps):
                        for kbi in range(2):
                            for hh in range(2):
                                h = 2 * hp + hh
                                nc.tensor.matmul(
                                    ot_ps[:, hh, :],
                                    lhsT=v_sb[:, 2 * bb + kbi, h, :],
                                    rhs=pT[:, hh, kbi, :],
                                    start=(bb == 0 and kbi == 0 and hh == 0),
                                    stop=(bb == nbat - 1 and kbi == 1 and hh == 1),
                                )

                    # software pipeline: PV for batch bb-1 is emitted after
                    # scores+exp of batch bb, so the in-order PE queue always
                    # has ready matmuls while ACT runs the exp
                    pending = []
                    for bb in range(nbat):
                        s_ps = ps_s.tile([P, 2, 2, QP], f32, tag="s")
                        for kbi in range(2):
                            kb = 2 * bb + kbi
                            for hh in range(2):
                                nc.tensor.matmul(
                                    s_ps[:, hh, kbi, :],
                                    lhsT=kT_sb[
                                        hh * DH : (hh + 1) * DH,
                                        hp,
                                        kb * P : (kb + 1) * P,
                                    ],
                                    rhs=qT_sb[
                                        hh * DH : (hh + 1) * DH, hp, q0 : q0 + QP
                                    ],
                                    start=True,
                                    stop=True,
                                )
                        pT = wpool.tile([P, 2, 2, QP], MMDT, tag="pT")
                        nc.scalar.activation(pT[:], s_ps[:], Exp, scale=SCALE)
                        for kbi in range(2):
                            kb = 2 * bb + kbi
                            if kb >= nblk - 4:
                                # zero masked probabilities; mask pre-expanded
                                # over the head dim -> no broadcast operand ->
                                # DVE 2x mode
                                i = kb - (nblk - 4)
                                if MASK_GS:
                                    eng = nc.gpsimd if kbi == 1 else nc.vector
                                else:
                                    eng = nc.vector
                                eng.tensor_tensor(
                                    pT[:, :, kbi, :],
                                    pT[:, :, kbi, :],
                                    mask_sb[:, p, i, :, :],
                                    mult_op,
                                )
                        pending.append((bb, pT))
                        if bb == norm_at and deferred["norm"] is not None:
                            deferred["norm"]()
                            deferred["norm"] = None
                        if bb == norm_at + 3 and deferred["o_mm"] is not None:
                            deferred["o_mm"]()
                            deferred["o_mm"] = None
                        if hp == 0 and bb == nbat - 1 and deferred["oproj"] is not None:
                            deferred["oproj"]()
                            deferred["oproj"] = None
                        if len(pending) > PVDEPTH:
                            emit_pv(*pending.pop(0))
                        # keep PE dense: pull next projection group for the
                        # following k-chunk while ACT digests this batch
                        if gen is not None:
                            next(gen, None)
                    for item in pending:
                        emit_pv(*item)
                    if deferred["o_mm"] is not None:
                        deferred["o_mm"]()
                        deferred["o_mm"] = None
                    if p == NPANEL - 1:
                        o_mm = make_o_mm(p, hp, attT_sb)
                        deferred["norm"] = make_norm(
                            hp,
                            ot_ps,
                            attT_sb,
                            after=(o_mm if hp == 3 else None),
                            split_hh=(hp == 3),
                        )
                        if hp < 3:
                            deferred["o_mm"] = o_mm
                    else:
                        deferred["norm"] = make_norm(hp, ot_ps, attT_sb)
                # drain any leftover projection groups of the next chunk
                if gen is not None:
                    for _ in gen:
                        pass
                if p < NPANEL - 1:
                    deferred["oproj"] = make_oproj(p, attT_sb)

            for _ in proj_chunk_gen(0):
                pass
            for p in range(NPANEL):
                gen = proj_chunk_gen(p + 1) if p < NPANEL - 1 else None
                emit_attention_panel(p, gen)
            deferred["norm"]()
    return nc


_NC = None


def _get_nc():
    global _NC
    if _NC is None:
        _NC = build()
        # run_bass_via_pjrt does not finalize; Bacc.finalize runs the compile
        # passes (register allocation, event-semaphore wait splitting).
        _NC.finalize()
    return _NC


def _qrows(half):
    return np.concatenate(
        [np.arange(512 * p + 256 * half, 512 * p + 256 * half + QP) for p in range(NPANEL)]
    )


def _mask_for(half, mmnp):
    m = np.empty((NPANEL, P, 4, 2, QP), np.float32)
    r = np.arange(P)[:, None]
    c = np.arange(QP)[None, :]
    for p in range(NPANEL):
        q0 = 512 * p + 256 * half
        for i in range(4):
            k = (4 * p + i) * P + r
            mi = np.where(k <= q0 + c, 1.0, 0.0)
            m[p, :, i, 0, :] = mi
            m[p, :, i, 1, :] = mi
    return np.ascontiguousarray(m).astype(mmnp)


def _pack_w(w):
    # [p, db, j] with contraction row d = db*128 + p
    return np.ascontiguousarray(w.T.reshape(4, P, D).transpose(1, 0, 2))


def _in_maps(inputs):
    mmnp = mybir.dt.np(MMDT)
    x = np.asarray(inputs["x"], np.float32)
    wq = _pack_w(np.asarray(inputs["W_Q_w"], np.float32)).astype(mmnp)
    wk = np.ascontiguousarray(
        np.asarray(inputs["W_K_w"], np.float32).T.reshape(4, P, D)
    ).astype(mmnp)
    wv = _pack_w(np.asarray(inputs["W_V_w"], np.float32)).astype(mmnp)
    woT = np.asarray(inputs["W_O_w"], np.float32).T  # [ (h,dh), n ]
    wo2 = np.ascontiguousarray(woT.reshape(4, P, D).transpose(1, 0, 2)).astype(mmnp)
    bq = np.ascontiguousarray(np.asarray(inputs["W_Q_b"], np.float32).reshape(4, P).T)
    # V bias folded through the O projection: bo' = bo + bv @ Wo.T
    bo_eff = np.asarray(inputs["W_O_b"], np.float32) + (
        np.asarray(inputs["W_V_b"], np.float32) @ woT
    )
    bo_bc = np.ascontiguousarray(np.broadcast_to(bo_eff, (P, D)))
    ones64 = np.ones((1, DH), mmnp)
    masks = [_mask_for(0, mmnp), _mask_for(1, mmnp)]
    in_maps = []
    for core in range(8):
        b, half = core // 2, core % 2
        xb = x[b]
        xT = xb.T  # [D, S]
        xTp = np.ascontiguousarray(
            xT.reshape(4, P, 4, 512).transpose(2, 1, 0, 3)
        ).astype(mmnp)
        xqT = xb[_qrows(half)].T  # [D, NQ]
        xqTp = np.ascontiguousarray(
            xqT.reshape(4, P, 2, 512).transpose(2, 1, 0, 3)
        ).astype(mmnp)
        in_maps.append(
            {
                "xTp": xTp,
                "xqTp": xqTp,
                "wqp": wq,
                "wkp": wk,
                "wvp": wv,
                "woT2": wo2,
                "bq": bq,
                "bo_bc": bo_bc,
                "maskp": masks[half],
                "ones64": ones64,
            }
        )
    return in_maps


def _assemble(results, B=4):
    out = np.empty((B, S, D), np.float32)
    for core in range(8):
        b, half = core // 2, core % 2
        out[b, _qrows(half), :] = results[core]["out"]
    return out


def run(inputs, trace=False, **kw):
    from concourse.bass_utils import run_bass_kernel_spmd

    res = run_bass_kernel_spmd(
        _get_nc(), _in_maps(inputs), core_ids=list(range(8)), trace=trace, **kw
    )
    return _assemble(res.results), res


def kernel(**inputs):
    out, _ = run(inputs, trace=False)
    return out

